# revision 32
# baseline (speedup 1.0000x reference)
"""Trainium2 Bass kernel for causal multi-head attention (dense transformer).

Problem shapes (hardcoded): x [2,2048,1024], 16 heads x 64 head-dim.
Sharding: data-parallel over batch (2) x tensor-parallel over heads (4/core)
on 8 NeuronCores. Each core computes the partial output (sum over its 4
heads) for one batch element; the host sums the 4 partials per batch and
adds b_O.

Per-core kernel (all matmuls float32r = full-rate fp32-rounded):
  - host passes x^T and pre-transposed weights, so no on-device transposes
  - scores computed as S^T[k,q] (k on partitions); causal mask applied by
    accumulating an identity x (lower-triangular -30000) matmul onto the
    diagonal 128x128 block; exp fused with PSUM->SBUF evacuation on ScalarE
  - AV uses V augmented with a ones column so the softmax denominator falls
    out of the same matmul; z^T is produced directly in out-proj layout
  - normalization: DVE fast reciprocal of the denominator row, DMA
    partition-broadcast, one tensor_tensor multiply per (head, q-chunk)
"""

import sys

if "/opt/trn_rl_repo" not in sys.path:
    sys.path.insert(0, "/opt/trn_rl_repo")

import numpy as np

B, S, D = 2, 2048, 1024
H, DH = 16, 64
NCORES = 8
NH = 4            # heads per core
KCH = D // 128    # contraction chunks over model dim
NT = S // 128     # 128-row tiles over sequence
QC = S // 512     # 512-wide q chunks
P = 128
MASK_VAL = -30000.0

_CACHE = {}


def _build_nc(debug=False):
    import concourse.tile as tile
    from concourse import bacc, mybir

    f32 = mybir.dt.float32
    f32r = mybir.dt.float32r
    bf16 = mybir.dt.bfloat16
    Exp = mybir.ActivationFunctionType.Exp
    mult = mybir.AluOpType.mult

    nc = bacc.Bacc("TRN2", target_bir_lowering=False, debug=False,
                   num_devices=NCORES)

    xt_d = nc.dram_tensor("xt", [D, S], f32, kind="ExternalInput").ap()
    wq_d = nc.dram_tensor("wq", [P, KCH * NH * DH], f32, kind="ExternalInput").ap()
    wk_d = nc.dram_tensor("wk", [P, KCH * NH * DH], f32, kind="ExternalInput").ap()
    wv_d = nc.dram_tensor("wv", [P, KCH * NH * DH], f32, kind="ExternalInput").ap()
    wo_d = nc.dram_tensor("wo", [P, 2 * D], f32, kind="ExternalInput").ap()
    bq_d = nc.dram_tensor("bq", [1, NH * DH], f32, kind="ExternalInput").ap()
    bk_d = nc.dram_tensor("bk", [1, NH * DH], f32, kind="ExternalInput").ap()
    bv_d = nc.dram_tensor("bv", [1, NH * DH], f32, kind="ExternalInput").ap()
    ones_d = nc.dram_tensor("ones", [1, S], f32, kind="ExternalInput").ap()
    zeros_d = nc.dram_tensor("zeros", [1, S], f32, kind="ExternalInput").ap()
    vones_d = nc.dram_tensor("vones", [P, NT * NH], f32, kind="ExternalInput").ap()
    tri_d = nc.dram_tensor("tri", [P, P], f32, kind="ExternalInput").ap()
    trim_d = nc.dram_tensor("trim", [P, P], f32, kind="ExternalInput").ap()
    iden_d = nc.dram_tensor("iden", [P, P], f32, kind="ExternalInput").ap()
    out_d = nc.dram_tensor("out", [S, D], f32, kind="ExternalOutput").ap()
    dbg = {}
    if debug:
        dbg["qt"] = nc.dram_tensor("dbg_qt", [P, 2 * S], f32, kind="ExternalOutput").ap()
        dbg["kt"] = nc.dram_tensor("dbg_kt", [P, NH * S], f32, kind="ExternalOutput").ap()
        dbg["v"] = nc.dram_tensor("dbg_v", [P, NT * NH * (DH + 1)], f32, kind="ExternalOutput").ap()
        dbg["zn"] = nc.dram_tensor("dbg_zn", [P, 2 * S], f32, kind="ExternalOutput").ap()
        dbg["es"] = nc.dram_tensor("dbg_es", [P, 1024], f32, kind="ExternalOutput").ap()
        dbg["av"] = nc.dram_tensor("dbg_av", [DH + 1, QC * 512], f32, kind="ExternalOutput").ap()
        dbg["rd"] = nc.dram_tensor("dbg_rd", [1, QC * 512], f32, kind="ExternalOutput").ap()
        dbg["rdb"] = nc.dram_tensor("dbg_rdb", [64, QC * 512], f32, kind="ExternalOutput").ap()

    with tile.TileContext(nc) as tc:
        from contextlib import ExitStack

        with ExitStack() as ctx:
            persist = ctx.enter_context(tc.tile_pool(name="persist", bufs=1))

            QT = persist.tile([P, 2, S], f32r)
            KT = persist.tile([P, NH, S], f32r)
            V = persist.tile([P, NT, NH, DH + 1], f32r)
            ZN = persist.tile([P, 2, S], f32r)
            WQ = persist.tile([P, KCH, NH * DH], f32r)
            WK = persist.tile([P, KCH, NH * DH], f32r)
            WV = persist.tile([P, KCH, NH * DH], f32r)
            WO = persist.tile([P, 2, D], f32r)
            BQ = persist.tile([1, NH * DH], f32r)
            BK = persist.tile([1, NH * DH], f32r)
            BV = persist.tile([1, NH * DH], f32r)
            ONES = persist.tile([1, S], f32r)
            TRI = persist.tile([P, P], f32)
            IDEN = persist.tile([P, P], bf16)


            # ---- input DMAs (gpsimd casts fp32 -> float32r in flight) ----
            nc.sync.dma_start(TRI, tri_d)
            nc.gpsimd.dma_start(IDEN, iden_d)

            nc.gpsimd.dma_start(BQ, bq_d)
            nc.gpsimd.dma_start(BK, bk_d)
            nc.gpsimd.dma_start(BV, bv_d)
            nc.gpsimd.dma_start(ONES, ones_d)
            nc.gpsimd.dma_start(WQ.rearrange("p a b -> p (a b)"), wq_d)
            nc.gpsimd.dma_start(WK.rearrange("p a b -> p (a b)"), wk_d)
            nc.gpsimd.dma_start(WV.rearrange("p a b -> p (a b)"), wv_d)
            nc.gpsimd.dma_start(V[:, :, :, DH:DH + 1], vones_d)
            nc.gpsimd.dma_start(WO.rearrange("p a b -> p (a b)"), wo_d)
            import concourse.bass as bass
            for h in range(NH):
                zb = (h % 2) * 64 ^ 64
                zsrc = bass.AP(tensor=zeros_d.tensor, offset=0,
                               ap=[[0, 64], [1, S]])
                nc.gpsimd.dma_start(KT[zb:zb + 64, h, :], zsrc)

            xt_pool = tc.tile_pool(name="xt", bufs=1)
            xt_ctx = xt_pool.__enter__()
            stg_pool = tc.tile_pool(name="stg", bufs=4)
            stg_ctx = stg_pool.__enter__()
            XT = xt_ctx.tile([P, KCH, S], f32r)
            for ch in range(KCH):
                for hh in range(2):
                    stg = stg_ctx.tile([P, 1024], f32, tag="stg",
                                       name=f"stg_{ch}_{hh}")
                    eng = nc.sync if (2 * ch + hh) % 2 == 0 else nc.scalar
                    eng.dma_start(stg, xt_d[ch * P:(ch + 1) * P,
                                            hh * 1024:(hh + 1) * 1024])
                    nc.gpsimd.tensor_copy(
                        out=XT[:, ch, hh * 1024:(hh + 1) * 1024], in_=stg)

            # ---- PE warmup: get HAM to K=8/8 while input DMAs stream ----
            with tc.tile_pool(name="warm_ps", bufs=1, space="PSUM") as warm_ps:
                wps = warm_ps.tile([P, P], mybir.dt.float32)
                for _ in range(36):
                    nc.tensor.matmul(wps, IDEN, IDEN, start=True, stop=True)

            # ---- phase 1: Q^T, K^T, V projections (chunk-major sweeps so
            # the PE starts as soon as the first x^T chunk lands) ----
            with tc.tile_pool(name="qkv_ps", bufs=8, space="PSUM") as qkv_ps:
                for sweep in range(2):           # qc pair (0,1) then (2,3)
                    pst = {}
                    for wi, (W_, B_) in enumerate(((WQ, BQ), (WK, BK))):
                        for t in range(2):
                            for qc in (2 * sweep, 2 * sweep + 1):
                                pst[(wi, t, qc)] = qkv_ps.tile(
                                    [P, 512], mybir.dt.float32, tag="qk",
                                    name=f"qk_{wi}_{t}_{qc}")
                    for ch in range(KCH):
                        for wi, (W_, B_) in enumerate(((WQ, BQ), (WK, BK))):
                            for t in range(2):
                                for qc in (2 * sweep, 2 * sweep + 1):
                                    nc.tensor.matmul(
                                        pst[(wi, t, qc)],
                                        W_[:, ch, t * P:(t + 1) * P],
                                        XT[:, ch, qc * 512:(qc + 1) * 512],
                                        start=(ch == 0), stop=False)
                    for wi, (W_, B_) in enumerate(((WQ, BQ), (WK, BK))):
                        for t in range(2):
                            for qc in (2 * sweep, 2 * sweep + 1):
                                ps = pst[(wi, t, qc)]
                                nc.tensor.matmul(
                                    ps, B_[:, t * P:(t + 1) * P],
                                    ONES[:, qc * 512:(qc + 1) * 512],
                                    start=False, stop=True)
                                sl = slice(qc * 512, (qc + 1) * 512)
                                if wi == 0:
                                    nc.vector.tensor_copy(QT[:, t, sl], ps)
                                else:
                                    nc.vector.tensor_copy(
                                        KT[0:64, 2 * t, sl], ps[0:64, :])
                                    nc.vector.tensor_copy(
                                        KT[64:128, 2 * t + 1, sl], ps[64:128, :])
                # V sweeps: 8 k-tiles at a time, chunk-major (same slots)
                for vs in range(2):
                    psv = [qkv_ps.tile([P, 512], mybir.dt.float32, tag="qk",
                                       name=f"v_{vs}_{i}") for i in range(KCH)]
                    for ch in range(KCH):
                        for i in range(KCH):
                            kt = vs * KCH + i
                            nc.tensor.matmul(
                                psv[i][:, 0:NH * DH],
                                XT[:, ch, kt * P:(kt + 1) * P],
                                WV[:, ch, :], start=(ch == 0), stop=False)
                    for i in range(KCH):
                        kt = vs * KCH + i
                        nc.tensor.matmul(
                            psv[i][:, 0:NH * DH],
                            ONES[:, kt * P:(kt + 1) * P], BV,
                            start=False, stop=True)
                        nc.vector.tensor_copy(V[:, kt, :, 0:DH], psv[i][:, 0:NH * DH])

            stg_pool.__exit__(None, None, None)
            xt_pool.__exit__(None, None, None)

            # ---- phase 2: attention; strips software-pipelined so the PE
            # emits scores(s+1) before AV(s) and never stalls on exp ----
            with tc.tile_pool(name="sc_ps", bufs=2, space="PSUM") as sc_ps, \
                    tc.tile_pool(name="av_ps", bufs=4, space="PSUM") as av_ps, \
                    tc.tile_pool(name="esp", bufs=4) as esp, \
                    tc.tile_pool(name="nrm", bufs=4) as nrm:
                avs = {}

                def emit_scores(h, kb, hf):
                    t, pb = h // 2, (h % 2) * 64
                    k0 = kb * P
                    hstart = hf * 1024
                    qstart = max(k0, hstart)
                    strip_ps = sc_ps.tile([P, 1024], mybir.dt.float32,
                                          name=f"sps_{h}_{kb}_{hf}", tag="sps")
                    strip_sb = esp.tile([P, 1024], f32r,
                                        name=f"ssb_{h}_{kb}_{hf}", tag="ssb")
                    has_diag = k0 >= hstart
                    qpos = qstart
                    while qpos < hstart + 1024:
                        qnext = min(hstart + 1024, (qpos // 512 + 1) * 512)
                        nc.tensor.matmul(
                            strip_ps[:, qpos - hstart:qnext - hstart],
                            KT[:, h, k0:k0 + P],
                            QT[:, t, qpos:qnext],
                            start=True, stop=True)
                        qpos = qnext
                    nc.scalar.activation(
                        strip_sb[:, qstart - hstart:1024],
                        strip_ps[:, qstart - hstart:1024], Exp)
                    if has_diag:
                        dsl = slice(k0 - hstart, k0 - hstart + P)
                        nc.vector.tensor_tensor(
                            strip_sb[:, dsl], strip_sb[:, dsl], TRI, mult)
                    if debug and h == 0 and kb == 0 and hf == 0:
                        nc.gpsimd.dma_start(dbg["es"], strip_sb)
                    return strip_sb

                def emit_av(h, kb, hf, strip_sb):
                    t, pb = h // 2, (h % 2) * 64
                    k0 = kb * P
                    hstart = hf * 1024
                    qstart = max(k0, hstart)
                    if kb == 0:
                        for qc in (2 * hf, 2 * hf + 1):
                            avs[(h, qc)] = av_ps.tile(
                                [DH + 1, 512], mybir.dt.float32,
                                tag="av", name=f"av_{h}_{qc}")
                    av = {qc: avs[(h, qc)] for qc in (2 * hf, 2 * hf + 1)}
                    qpos = qstart
                    while qpos < hstart + 1024:
                        qc = qpos // 512
                        qnext = min(hstart + 1024, (qc + 1) * 512)
                        done = kb == 4 * qc + 3
                        nc.tensor.matmul(
                            av[qc][:, qpos - qc * 512:qnext - qc * 512],
                            V[:, kb, h, :],
                            strip_sb[:, qpos - hstart:qnext - hstart],
                            start=(kb == 0), stop=done)
                        if done:
                            emit_norm(h, qc, av[qc])
                        qpos = qnext

                def emit_norm(h, qc, avq):
                    t, pb = h // 2, (h % 2) * 64
                    if debug and h == 0:
                        avc = nrm.tile([DH + 1, 512], mybir.dt.float32,
                                       tag="avc", name=f"avc_{qc}")
                        nc.vector.tensor_copy(avc, avq)
                        nc.sync.dma_start(
                            dbg["av"][:, qc * 512:(qc + 1) * 512], avc)
                    rd = nrm.tile([1, 512], mybir.dt.float32, tag="rd")
                    nc.vector.tensor_copy(rd, avq[DH:DH + 1, :])
                    rr = nrm.tile([1, 512], mybir.dt.float32, tag="rr")
                    nc.vector.reciprocal_approx_fast(out=rr, in_=rd)
                    rdb = nrm.tile([64, 512], mybir.dt.float32, tag="rdb")
                    nc.gpsimd.partition_broadcast(rdb, rr)
                    if debug and h == 0:
                        nc.sync.dma_start(
                            dbg["rd"][:, qc * 512:(qc + 1) * 512], rr)
                        nc.sync.dma_start(
                            dbg["rdb"][:, qc * 512:(qc + 1) * 512], rdb)
                    zslc = ZN[pb:pb + 64, t, qc * 512:(qc + 1) * 512]
                    nc.vector.tensor_tensor(zslc, avq[0:DH, :], rdb, mult)

                strip_ids = []
                for h in range(NH):
                    for hf in range(2):
                        for kb in range(NT):
                            if hf * 1024 + 1024 > kb * P:
                                strip_ids.append((h, kb, hf))
                from collections import deque
                pending = deque()
                for sid in strip_ids:
                    sb_tile = emit_scores(*sid)
                    pending.append((sid, sb_tile))
                    if len(pending) > 4:
                        psid, psb = pending.popleft()
                        emit_av(*psid, psb)
                while pending:
                    psid, psb = pending.popleft()
                    emit_av(*psid, psb)

            if debug:
                for nm, tl in (("qt", QT), ("kt", KT), ("v", V), ("zn", ZN)):
                    nc.gpsimd.dma_start(dbg[nm], tl.rearrange("p ... -> p (...)"))

            # ---- phase 3: output projection ----
            with tc.tile_pool(name="op_ps", bufs=3, space="PSUM") as op_ps, \
                    tc.tile_pool(name="osb", bufs=3) as osb:
                for qt in range(NT):
                    for dc in range(2):
                        ps = op_ps.tile([P, 512], mybir.dt.float32)
                        for t in range(2):
                            nc.tensor.matmul(
                                ps, ZN[:, t, qt * P:(qt + 1) * P],
                                WO[:, t, dc * 512:(dc + 1) * 512],
                                start=(t == 0), stop=(t == 1))
                        ob = osb.tile([P, 512], mybir.dt.float32)
                        if (qt + dc) % 2 == 0:
                            nc.scalar.copy(ob, ps)
                        else:
                            nc.vector.tensor_copy(ob, ps)
                        oeng = (nc.sync, nc.scalar, nc.gpsimd)[(2 * qt + dc) % 3]
                        oeng.dma_start(
                            out_d[qt * P:(qt + 1) * P, dc * 512:(dc + 1) * 512],
                            ob)

    nc.compile()
    return nc


def _get_nc(debug=False):
    key = ("nc", debug)
    if key not in _CACHE:
        _CACHE[key] = _build_nc(debug)
    return _CACHE[key]


def _host_inputs(x, W_Q, W_K, W_V, W_O, b_Q, b_K, b_V):
    """Build the 8 per-core input maps."""
    x = np.asarray(x, dtype=np.float32)
    scale = 1.0 / np.sqrt(np.float32(DH))
    ones = np.ones((1, S), dtype=np.float32)
    vones = np.ones((P, NT * NH), dtype=np.float32)
    tri = (np.arange(P)[:, None] <= np.arange(P)[None, :]).astype(np.float32)
    trim = np.where(np.arange(P)[:, None] <= np.arange(P)[None, :],
                    np.float32(0.0), np.float32(MASK_VAL)).astype(np.float32)
    iden = np.eye(P, dtype=np.float32)

    xts = [np.ascontiguousarray(x[b].T) for b in range(B)]

    in_maps = []
    for c in range(NCORES):
        b, hg = divmod(c, NCORES // B)
        h0 = NH * hg
        def chunked(a):   # [D, M] -> [128, KCH*M] with rows p, cols (ch, m)
            return np.ascontiguousarray(
                a.reshape(KCH, P, -1).transpose(1, 0, 2).reshape(P, -1))
        wq = chunked((np.asarray(W_Q[h0:h0 + NH], np.float32) * scale)
                     .reshape(NH * DH, D).T)
        wk = chunked(np.asarray(W_K[h0:h0 + NH], np.float32)
                     .reshape(NH * DH, D).T)
        wv = chunked(np.asarray(W_V[h0:h0 + NH], np.float32)
                     .reshape(NH * DH, D).T)
        wo_flat = np.asarray(W_O[h0:h0 + NH], np.float32) \
            .transpose(0, 2, 1).reshape(NH * DH, D)
        wo = np.ascontiguousarray(
            wo_flat.reshape(2, P, D).transpose(1, 0, 2).reshape(P, 2 * D))
        bq = (np.asarray(b_Q[h0:h0 + NH], np.float32) * scale).reshape(1, NH * DH)
        bk = np.asarray(b_K[h0:h0 + NH], np.float32).reshape(1, NH * DH)
        bv = np.asarray(b_V[h0:h0 + NH], np.float32).reshape(1, NH * DH)
        in_maps.append({
            "xt": xts[b], "wq": wq, "wk": wk, "wv": wv, "wo": wo,
            "zeros": np.zeros((1, S), np.float32),
            "bq": np.ascontiguousarray(bq), "bk": np.ascontiguousarray(bk),
            "bv": np.ascontiguousarray(bv), "ones": ones, "vones": vones,
            "tri": tri, "trim": trim, "iden": iden,
        })
    return in_maps


def run_spmd(in_maps, debug=False, **kwargs):
    from concourse import bass_utils
    nc = _get_nc(debug)
    return bass_utils.run_bass_kernel_spmd(
        nc, in_maps, core_ids=list(range(NCORES)), **kwargs)


def kernel(x, W_Q, W_K, W_V, W_O, b_Q, b_K, b_V, b_O):
    in_maps = _host_inputs(x, W_Q, W_K, W_V, W_O, b_Q, b_K, b_V)
    res = run_spmd(in_maps)
    parts = [res.results[c]["out"] for c in range(NCORES)]
    gpb = NCORES // B
    out = np.stack(
        [sum(parts[b * gpb + g] for g in range(gpb)) for b in range(B)], axis=0)
    out += np.asarray(b_O, np.float32)[None, None, :]
    return out.astype(np.float32)


# revision 33
# speedup vs baseline: 1.0862x; 1.0862x over previous
"""Trainium2 Bass kernel for causal multi-head attention (dense transformer).

Problem shapes (hardcoded): x [2,2048,1024], 16 heads x 64 head-dim.
Sharding: data-parallel over batch (2) x tensor-parallel over heads (4/core)
on 8 NeuronCores. Each core computes the partial output (sum over its 4
heads) for one batch element; the host sums the 4 partials per batch and
adds b_O.

Per-core kernel (all matmuls float32r = full-rate fp32-rounded):
  - host passes x^T and pre-transposed weights, so no on-device transposes
  - scores computed as S^T[k,q] (k on partitions); causal mask applied by
    accumulating an identity x (lower-triangular -30000) matmul onto the
    diagonal 128x128 block; exp fused with PSUM->SBUF evacuation on ScalarE
  - AV uses V augmented with a ones column so the softmax denominator falls
    out of the same matmul; z^T is produced directly in out-proj layout
  - normalization: DVE fast reciprocal of the denominator row, DMA
    partition-broadcast, one tensor_tensor multiply per (head, q-chunk)
"""

import sys

if "/opt/trn_rl_repo" not in sys.path:
    sys.path.insert(0, "/opt/trn_rl_repo")

import numpy as np

B, S, D = 2, 2048, 1024
H, DH = 16, 64
NCORES = 8
NH = 4            # heads per core
KCH = D // 128    # contraction chunks over model dim
NT = S // 128     # 128-row tiles over sequence
QC = S // 512     # 512-wide q chunks
P = 128
MASK_VAL = -30000.0

_CACHE = {}


def _build_nc(debug=False):
    import concourse.tile as tile
    from concourse import bacc, mybir

    f32 = mybir.dt.float32
    f32r = mybir.dt.float32r
    bf16 = mybir.dt.bfloat16
    Exp = mybir.ActivationFunctionType.Exp
    mult = mybir.AluOpType.mult

    nc = bacc.Bacc("TRN2", target_bir_lowering=False, debug=False,
                   num_devices=NCORES)

    xt_d = nc.dram_tensor("xt", [D, S], f32, kind="ExternalInput").ap()
    wq_d = nc.dram_tensor("wq", [P, KCH * NH * DH], f32, kind="ExternalInput").ap()
    wk_d = nc.dram_tensor("wk", [P, KCH * NH * DH], f32, kind="ExternalInput").ap()
    wv_d = nc.dram_tensor("wv", [P, KCH * NH * DH], f32, kind="ExternalInput").ap()
    wo_d = nc.dram_tensor("wo", [P, 2 * D], f32, kind="ExternalInput").ap()
    bq_d = nc.dram_tensor("bq", [1, NH * DH], f32, kind="ExternalInput").ap()
    bk_d = nc.dram_tensor("bk", [1, NH * DH], f32, kind="ExternalInput").ap()
    bv_d = nc.dram_tensor("bv", [1, NH * DH], f32, kind="ExternalInput").ap()
    ones_d = nc.dram_tensor("ones", [1, S], f32, kind="ExternalInput").ap()
    zeros_d = nc.dram_tensor("zeros", [1, S], f32, kind="ExternalInput").ap()
    vones_d = nc.dram_tensor("vones", [P, NT * NH], f32, kind="ExternalInput").ap()
    tri_d = nc.dram_tensor("tri", [P, P], f32, kind="ExternalInput").ap()
    trim_d = nc.dram_tensor("trim", [P, P], f32, kind="ExternalInput").ap()
    iden_d = nc.dram_tensor("iden", [P, P], f32, kind="ExternalInput").ap()
    out_d = nc.dram_tensor("out", [S, D], f32, kind="ExternalOutput").ap()
    dbg = {}
    if debug:
        dbg["qt"] = nc.dram_tensor("dbg_qt", [P, 2 * S], f32, kind="ExternalOutput").ap()
        dbg["kt"] = nc.dram_tensor("dbg_kt", [P, NH * S], f32, kind="ExternalOutput").ap()
        dbg["v"] = nc.dram_tensor("dbg_v", [P, NT * NH * (DH + 1)], f32, kind="ExternalOutput").ap()
        dbg["zn"] = nc.dram_tensor("dbg_zn", [P, 2 * S], f32, kind="ExternalOutput").ap()
        dbg["es"] = nc.dram_tensor("dbg_es", [P, 1024], f32, kind="ExternalOutput").ap()
        dbg["av"] = nc.dram_tensor("dbg_av", [DH + 1, QC * 512], f32, kind="ExternalOutput").ap()
        dbg["rd"] = nc.dram_tensor("dbg_rd", [1, QC * 512], f32, kind="ExternalOutput").ap()
        dbg["rdb"] = nc.dram_tensor("dbg_rdb", [64, QC * 512], f32, kind="ExternalOutput").ap()

    with tile.TileContext(nc) as tc:
        from contextlib import ExitStack

        with ExitStack() as ctx:
            persist = ctx.enter_context(tc.tile_pool(name="persist", bufs=1))

            QT = persist.tile([P, 2, S], f32r)
            KT = persist.tile([P, NH, S], f32r)
            V = persist.tile([P, NT, NH, DH + 1], f32r)
            ZN = persist.tile([P, 2, S], f32r)
            WQ = persist.tile([P, KCH, NH * DH], f32r)
            WK = persist.tile([P, KCH, NH * DH], f32r)
            WV = persist.tile([P, KCH, NH * DH], f32r)
            WO = persist.tile([P, 2, D], f32r)
            BQ = persist.tile([1, NH * DH], f32r)
            BK = persist.tile([1, NH * DH], f32r)
            BV = persist.tile([1, NH * DH], f32r)
            ONES = persist.tile([1, S], f32r)
            TRI = persist.tile([P, P], f32)
            IDEN = persist.tile([P, P], bf16)


            # ---- input DMAs (gpsimd casts fp32 -> float32r in flight) ----
            nc.sync.dma_start(TRI, tri_d)
            nc.gpsimd.dma_start(IDEN, iden_d)

            nc.gpsimd.dma_start(BQ, bq_d)
            nc.gpsimd.dma_start(BK, bk_d)
            nc.gpsimd.dma_start(BV, bv_d)
            nc.gpsimd.dma_start(ONES, ones_d)
            nc.gpsimd.dma_start(WQ.rearrange("p a b -> p (a b)"), wq_d)
            nc.gpsimd.dma_start(WK.rearrange("p a b -> p (a b)"), wk_d)
            nc.gpsimd.dma_start(WV.rearrange("p a b -> p (a b)"), wv_d)
            nc.gpsimd.dma_start(V[:, :, :, DH:DH + 1], vones_d)
            nc.gpsimd.dma_start(WO.rearrange("p a b -> p (a b)"), wo_d)
            import concourse.bass as bass
            for h in range(NH):
                zb = (h % 2) * 64 ^ 64
                zsrc = bass.AP(tensor=zeros_d.tensor, offset=0,
                               ap=[[0, 64], [1, S]])
                nc.gpsimd.dma_start(KT[zb:zb + 64, h, :], zsrc)

            xt_pool = tc.tile_pool(name="xt", bufs=1)
            xt_ctx = xt_pool.__enter__()
            stg_pool = tc.tile_pool(name="stg", bufs=4)
            stg_ctx = stg_pool.__enter__()
            XT = xt_ctx.tile([P, KCH, S], f32r)
            for ch in range(KCH):
                for hh in range(2):
                    stg = stg_ctx.tile([P, 1024], f32, tag="stg",
                                       name=f"stg_{ch}_{hh}")
                    eng = nc.sync if (2 * ch + hh) % 2 == 0 else nc.scalar
                    eng.dma_start(stg, xt_d[ch * P:(ch + 1) * P,
                                            hh * 1024:(hh + 1) * 1024])
                    nc.vector.tensor_copy(
                        XT[:, ch, hh * 1024:(hh + 1) * 1024], stg)

            # ---- PE warmup: get HAM to K=8/8 while input DMAs stream ----
            with tc.tile_pool(name="warm_ps", bufs=1, space="PSUM") as warm_ps:
                wps = warm_ps.tile([P, P], mybir.dt.float32)
                for _ in range(36):
                    nc.tensor.matmul(wps, IDEN, IDEN, start=True, stop=True)

            # ---- phase 1: Q^T, K^T, V projections (chunk-major sweeps so
            # the PE starts as soon as the first x^T chunk lands) ----
            with tc.tile_pool(name="qkv_ps", bufs=8, space="PSUM") as qkv_ps:
                for sweep in range(2):           # qc pair (0,1) then (2,3)
                    pst = {}
                    for wi, (W_, B_) in enumerate(((WQ, BQ), (WK, BK))):
                        for t in range(2):
                            for qc in (2 * sweep, 2 * sweep + 1):
                                pst[(wi, t, qc)] = qkv_ps.tile(
                                    [P, 512], mybir.dt.float32, tag="qk",
                                    name=f"qk_{wi}_{t}_{qc}")
                    for ch in range(KCH):
                        for wi, (W_, B_) in enumerate(((WQ, BQ), (WK, BK))):
                            for t in range(2):
                                for qc in (2 * sweep, 2 * sweep + 1):
                                    nc.tensor.matmul(
                                        pst[(wi, t, qc)],
                                        W_[:, ch, t * P:(t + 1) * P],
                                        XT[:, ch, qc * 512:(qc + 1) * 512],
                                        start=(ch == 0), stop=False)
                    for wi, (W_, B_) in enumerate(((WQ, BQ), (WK, BK))):
                        for t in range(2):
                            for qc in (2 * sweep, 2 * sweep + 1):
                                ps = pst[(wi, t, qc)]
                                nc.tensor.matmul(
                                    ps, B_[:, t * P:(t + 1) * P],
                                    ONES[:, qc * 512:(qc + 1) * 512],
                                    start=False, stop=True)
                                sl = slice(qc * 512, (qc + 1) * 512)
                                if wi == 0:
                                    nc.vector.tensor_copy(QT[:, t, sl], ps)
                                else:
                                    nc.vector.tensor_copy(
                                        KT[0:64, 2 * t, sl], ps[0:64, :])
                                    nc.vector.tensor_copy(
                                        KT[64:128, 2 * t + 1, sl], ps[64:128, :])
                # V sweeps: 8 k-tiles at a time, chunk-major (same slots)
                for vs in range(2):
                    psv = [qkv_ps.tile([P, 512], mybir.dt.float32, tag="qk",
                                       name=f"v_{vs}_{i}") for i in range(KCH)]
                    for ch in range(KCH):
                        for i in range(KCH):
                            kt = vs * KCH + i
                            nc.tensor.matmul(
                                psv[i][:, 0:NH * DH],
                                XT[:, ch, kt * P:(kt + 1) * P],
                                WV[:, ch, :], start=(ch == 0), stop=False)
                    for i in range(KCH):
                        kt = vs * KCH + i
                        nc.tensor.matmul(
                            psv[i][:, 0:NH * DH],
                            ONES[:, kt * P:(kt + 1) * P], BV,
                            start=False, stop=True)
                        nc.vector.tensor_copy(V[:, kt, :, 0:DH], psv[i][:, 0:NH * DH])

            stg_pool.__exit__(None, None, None)
            xt_pool.__exit__(None, None, None)

            # ---- phase 2: attention; strips software-pipelined so the PE
            # emits scores(s+1) before AV(s) and never stalls on exp ----
            with tc.tile_pool(name="sc_ps", bufs=2, space="PSUM") as sc_ps, \
                    tc.tile_pool(name="av_ps", bufs=4, space="PSUM") as av_ps, \
                    tc.tile_pool(name="esp", bufs=4) as esp, \
                    tc.tile_pool(name="nrm", bufs=4) as nrm:
                avs = {}

                def emit_scores(h, kb, hf):
                    t, pb = h // 2, (h % 2) * 64
                    k0 = kb * P
                    hstart = hf * 1024
                    qstart = max(k0, hstart)
                    strip_ps = sc_ps.tile([P, 1024], mybir.dt.float32,
                                          name=f"sps_{h}_{kb}_{hf}", tag="sps")
                    strip_sb = esp.tile([P, 1024], f32r,
                                        name=f"ssb_{h}_{kb}_{hf}", tag="ssb")
                    has_diag = k0 >= hstart
                    qpos = qstart
                    while qpos < hstart + 1024:
                        qnext = min(hstart + 1024, (qpos // 512 + 1) * 512)
                        nc.tensor.matmul(
                            strip_ps[:, qpos - hstart:qnext - hstart],
                            KT[:, h, k0:k0 + P],
                            QT[:, t, qpos:qnext],
                            start=True, stop=True)
                        qpos = qnext
                    nc.scalar.activation(
                        strip_sb[:, qstart - hstart:1024],
                        strip_ps[:, qstart - hstart:1024], Exp)
                    if has_diag:
                        dsl = slice(k0 - hstart, k0 - hstart + P)
                        nc.vector.tensor_tensor(
                            strip_sb[:, dsl], strip_sb[:, dsl], TRI, mult)
                    if debug and h == 0 and kb == 0 and hf == 0:
                        nc.gpsimd.dma_start(dbg["es"], strip_sb)
                    return strip_sb

                def emit_av(h, kb, hf, strip_sb):
                    t, pb = h // 2, (h % 2) * 64
                    k0 = kb * P
                    hstart = hf * 1024
                    qstart = max(k0, hstart)
                    if kb == 0:
                        for qc in (2 * hf, 2 * hf + 1):
                            avs[(h, qc)] = av_ps.tile(
                                [DH + 1, 512], mybir.dt.float32,
                                tag="av", name=f"av_{h}_{qc}")
                    av = {qc: avs[(h, qc)] for qc in (2 * hf, 2 * hf + 1)}
                    qpos = qstart
                    while qpos < hstart + 1024:
                        qc = qpos // 512
                        qnext = min(hstart + 1024, (qc + 1) * 512)
                        done = kb == 4 * qc + 3
                        nc.tensor.matmul(
                            av[qc][:, qpos - qc * 512:qnext - qc * 512],
                            V[:, kb, h, :],
                            strip_sb[:, qpos - hstart:qnext - hstart],
                            start=(kb == 0), stop=done)
                        if done:
                            emit_norm(h, qc, av[qc])
                        qpos = qnext

                def emit_norm(h, qc, avq):
                    t, pb = h // 2, (h % 2) * 64
                    if debug and h == 0:
                        avc = nrm.tile([DH + 1, 512], mybir.dt.float32,
                                       tag="avc", name=f"avc_{qc}")
                        nc.vector.tensor_copy(avc, avq)
                        nc.sync.dma_start(
                            dbg["av"][:, qc * 512:(qc + 1) * 512], avc)
                    rd = nrm.tile([1, 512], mybir.dt.float32, tag="rd")
                    nc.vector.tensor_copy(rd, avq[DH:DH + 1, :])
                    rr = nrm.tile([1, 512], mybir.dt.float32, tag="rr")
                    nc.vector.reciprocal_approx_fast(out=rr, in_=rd)
                    rdb = nrm.tile([64, 512], mybir.dt.float32, tag="rdb")
                    nc.gpsimd.partition_broadcast(rdb, rr)
                    if debug and h == 0:
                        nc.sync.dma_start(
                            dbg["rd"][:, qc * 512:(qc + 1) * 512], rr)
                        nc.sync.dma_start(
                            dbg["rdb"][:, qc * 512:(qc + 1) * 512], rdb)
                    zslc = ZN[pb:pb + 64, t, qc * 512:(qc + 1) * 512]
                    nc.vector.tensor_tensor(zslc, avq[0:DH, :], rdb, mult)

                strip_ids = []
                for h in range(NH):
                    for hf in range(2):
                        for kb in range(NT):
                            if hf * 1024 + 1024 > kb * P:
                                strip_ids.append((h, kb, hf))
                from collections import deque
                pending = deque()
                for sid in strip_ids:
                    sb_tile = emit_scores(*sid)
                    pending.append((sid, sb_tile))
                    if len(pending) > 4:
                        psid, psb = pending.popleft()
                        emit_av(*psid, psb)
                while pending:
                    psid, psb = pending.popleft()
                    emit_av(*psid, psb)

            if debug:
                for nm, tl in (("qt", QT), ("kt", KT), ("v", V), ("zn", ZN)):
                    nc.gpsimd.dma_start(dbg[nm], tl.rearrange("p ... -> p (...)"))

            # ---- phase 3: output projection ----
            with tc.tile_pool(name="op_ps", bufs=3, space="PSUM") as op_ps, \
                    tc.tile_pool(name="osb", bufs=3) as osb:
                for qt in range(NT):
                    for dc in range(2):
                        ps = op_ps.tile([P, 512], mybir.dt.float32)
                        for t in range(2):
                            nc.tensor.matmul(
                                ps, ZN[:, t, qt * P:(qt + 1) * P],
                                WO[:, t, dc * 512:(dc + 1) * 512],
                                start=(t == 0), stop=(t == 1))
                        ob = osb.tile([P, 512], mybir.dt.float32)
                        if (qt + dc) % 2 == 0:
                            nc.scalar.copy(ob, ps)
                        else:
                            nc.vector.tensor_copy(ob, ps)
                        oeng = (nc.sync, nc.scalar, nc.gpsimd)[(2 * qt + dc) % 3]
                        oeng.dma_start(
                            out_d[qt * P:(qt + 1) * P, dc * 512:(dc + 1) * 512],
                            ob)

    nc.compile()
    return nc


def _get_nc(debug=False):
    key = ("nc", debug)
    if key not in _CACHE:
        _CACHE[key] = _build_nc(debug)
    return _CACHE[key]


def _host_inputs(x, W_Q, W_K, W_V, W_O, b_Q, b_K, b_V):
    """Build the 8 per-core input maps."""
    x = np.asarray(x, dtype=np.float32)
    scale = 1.0 / np.sqrt(np.float32(DH))
    ones = np.ones((1, S), dtype=np.float32)
    vones = np.ones((P, NT * NH), dtype=np.float32)
    tri = (np.arange(P)[:, None] <= np.arange(P)[None, :]).astype(np.float32)
    trim = np.where(np.arange(P)[:, None] <= np.arange(P)[None, :],
                    np.float32(0.0), np.float32(MASK_VAL)).astype(np.float32)
    iden = np.eye(P, dtype=np.float32)

    xts = [np.ascontiguousarray(x[b].T) for b in range(B)]

    in_maps = []
    for c in range(NCORES):
        b, hg = divmod(c, NCORES // B)
        h0 = NH * hg
        def chunked(a):   # [D, M] -> [128, KCH*M] with rows p, cols (ch, m)
            return np.ascontiguousarray(
                a.reshape(KCH, P, -1).transpose(1, 0, 2).reshape(P, -1))
        wq = chunked((np.asarray(W_Q[h0:h0 + NH], np.float32) * scale)
                     .reshape(NH * DH, D).T)
        wk = chunked(np.asarray(W_K[h0:h0 + NH], np.float32)
                     .reshape(NH * DH, D).T)
        wv = chunked(np.asarray(W_V[h0:h0 + NH], np.float32)
                     .reshape(NH * DH, D).T)
        wo_flat = np.asarray(W_O[h0:h0 + NH], np.float32) \
            .transpose(0, 2, 1).reshape(NH * DH, D)
        wo = np.ascontiguousarray(
            wo_flat.reshape(2, P, D).transpose(1, 0, 2).reshape(P, 2 * D))
        bq = (np.asarray(b_Q[h0:h0 + NH], np.float32) * scale).reshape(1, NH * DH)
        bk = np.asarray(b_K[h0:h0 + NH], np.float32).reshape(1, NH * DH)
        bv = np.asarray(b_V[h0:h0 + NH], np.float32).reshape(1, NH * DH)
        in_maps.append({
            "xt": xts[b], "wq": wq, "wk": wk, "wv": wv, "wo": wo,
            "zeros": np.zeros((1, S), np.float32),
            "bq": np.ascontiguousarray(bq), "bk": np.ascontiguousarray(bk),
            "bv": np.ascontiguousarray(bv), "ones": ones, "vones": vones,
            "tri": tri, "trim": trim, "iden": iden,
        })
    return in_maps


def run_spmd(in_maps, debug=False, **kwargs):
    from concourse import bass_utils
    nc = _get_nc(debug)
    return bass_utils.run_bass_kernel_spmd(
        nc, in_maps, core_ids=list(range(NCORES)), **kwargs)


def kernel(x, W_Q, W_K, W_V, W_O, b_Q, b_K, b_V, b_O):
    in_maps = _host_inputs(x, W_Q, W_K, W_V, W_O, b_Q, b_K, b_V)
    res = run_spmd(in_maps)
    parts = [res.results[c]["out"] for c in range(NCORES)]
    gpb = NCORES // B
    out = np.stack(
        [sum(parts[b * gpb + g] for g in range(gpb)) for b in range(B)], axis=0)
    out += np.asarray(b_O, np.float32)[None, None, :]
    return out.astype(np.float32)


# revision 34
# speedup vs baseline: 1.1174x; 1.0287x over previous
"""Trainium2 Bass kernel for causal multi-head attention (dense transformer).

Problem shapes (hardcoded): x [2,2048,1024], 16 heads x 64 head-dim.
Sharding: data-parallel over batch (2) x tensor-parallel over heads (4/core)
on 8 NeuronCores. Each core computes the partial output (sum over its 4
heads) for one batch element; the host sums the 4 partials per batch and
adds b_O.

Per-core kernel (all matmuls float32r = full-rate fp32-rounded):
  - host passes x^T and pre-transposed weights, so no on-device transposes
  - scores computed as S^T[k,q] (k on partitions); causal mask applied by
    accumulating an identity x (lower-triangular -30000) matmul onto the
    diagonal 128x128 block; exp fused with PSUM->SBUF evacuation on ScalarE
  - AV uses V augmented with a ones column so the softmax denominator falls
    out of the same matmul; z^T is produced directly in out-proj layout
  - normalization: DVE fast reciprocal of the denominator row, DMA
    partition-broadcast, one tensor_tensor multiply per (head, q-chunk)
"""

import sys

if "/opt/trn_rl_repo" not in sys.path:
    sys.path.insert(0, "/opt/trn_rl_repo")

import numpy as np

B, S, D = 2, 2048, 1024
H, DH = 16, 64
NCORES = 8
NH = 4            # heads per core
KCH = D // 128    # contraction chunks over model dim
NT = S // 128     # 128-row tiles over sequence
QC = S // 512     # 512-wide q chunks
P = 128
MASK_VAL = -30000.0

_CACHE = {}


def _build_nc(debug=False):
    import concourse.tile as tile
    from concourse import bacc, mybir

    f32 = mybir.dt.float32
    f32r = mybir.dt.float32r
    bf16 = mybir.dt.bfloat16
    Exp = mybir.ActivationFunctionType.Exp
    mult = mybir.AluOpType.mult

    nc = bacc.Bacc("TRN2", target_bir_lowering=False, debug=False,
                   num_devices=NCORES)

    xt_d = nc.dram_tensor("xt", [D, S], f32, kind="ExternalInput").ap()
    wq_d = nc.dram_tensor("wq", [P, KCH * NH * DH], f32, kind="ExternalInput").ap()
    wk_d = nc.dram_tensor("wk", [P, KCH * NH * DH], f32, kind="ExternalInput").ap()
    wv_d = nc.dram_tensor("wv", [P, KCH * NH * DH], f32, kind="ExternalInput").ap()
    wo_d = nc.dram_tensor("wo", [P, 2 * D], f32, kind="ExternalInput").ap()
    bq_d = nc.dram_tensor("bq", [1, NH * DH], f32, kind="ExternalInput").ap()
    bk_d = nc.dram_tensor("bk", [1, NH * DH], f32, kind="ExternalInput").ap()
    bv_d = nc.dram_tensor("bv", [1, NH * DH], f32, kind="ExternalInput").ap()
    ones_d = nc.dram_tensor("ones", [1, S], f32, kind="ExternalInput").ap()
    zeros_d = nc.dram_tensor("zeros", [1, S], f32, kind="ExternalInput").ap()
    vones_d = nc.dram_tensor("vones", [P, NT * NH], f32, kind="ExternalInput").ap()
    tri_d = nc.dram_tensor("tri", [P, P], f32, kind="ExternalInput").ap()
    trim_d = nc.dram_tensor("trim", [P, P], f32, kind="ExternalInput").ap()
    iden_d = nc.dram_tensor("iden", [P, P], f32, kind="ExternalInput").ap()
    out_d = nc.dram_tensor("out", [S, D], f32, kind="ExternalOutput").ap()
    dbg = {}
    if debug:
        dbg["qt"] = nc.dram_tensor("dbg_qt", [P, 2 * S], f32, kind="ExternalOutput").ap()
        dbg["kt"] = nc.dram_tensor("dbg_kt", [P, NH * S], f32, kind="ExternalOutput").ap()
        dbg["v"] = nc.dram_tensor("dbg_v", [P, NT * NH * (DH + 1)], f32, kind="ExternalOutput").ap()
        dbg["zn"] = nc.dram_tensor("dbg_zn", [P, 2 * S], f32, kind="ExternalOutput").ap()
        dbg["es"] = nc.dram_tensor("dbg_es", [P, 1024], f32, kind="ExternalOutput").ap()
        dbg["av"] = nc.dram_tensor("dbg_av", [DH + 1, QC * 512], f32, kind="ExternalOutput").ap()
        dbg["rd"] = nc.dram_tensor("dbg_rd", [1, QC * 512], f32, kind="ExternalOutput").ap()
        dbg["rdb"] = nc.dram_tensor("dbg_rdb", [64, QC * 512], f32, kind="ExternalOutput").ap()

    with tile.TileContext(nc) as tc:
        from contextlib import ExitStack

        with ExitStack() as ctx:
            persist = ctx.enter_context(tc.tile_pool(name="persist", bufs=1))

            QT = persist.tile([P, 2, S], f32r)
            KT = persist.tile([P, NH, S], f32r)
            V = persist.tile([P, NT, NH, DH + 1], f32r)
            ZN = persist.tile([P, 2, S], f32r)
            WQ = persist.tile([P, KCH, NH * DH], f32r)
            WK = persist.tile([P, KCH, NH * DH], f32r)
            WV = persist.tile([P, KCH, NH * DH], f32r)
            WO = persist.tile([P, 2, D], f32r)
            BQ = persist.tile([1, NH * DH], f32r)
            BK = persist.tile([1, NH * DH], f32r)
            BV = persist.tile([1, NH * DH], f32r)
            ONES = persist.tile([1, S], f32r)
            TRI = persist.tile([P, P], f32)
            IDEN = persist.tile([P, P], bf16)


            # ---- input DMAs (gpsimd casts fp32 -> float32r in flight) ----
            nc.sync.dma_start(TRI, tri_d)
            nc.gpsimd.dma_start(IDEN, iden_d)

            nc.gpsimd.dma_start(BQ, bq_d)
            nc.gpsimd.dma_start(BK, bk_d)
            nc.gpsimd.dma_start(BV, bv_d)
            nc.gpsimd.dma_start(ONES, ones_d)
            nc.gpsimd.dma_start(WQ.rearrange("p a b -> p (a b)"), wq_d)
            nc.gpsimd.dma_start(WK.rearrange("p a b -> p (a b)"), wk_d)
            nc.gpsimd.dma_start(WV.rearrange("p a b -> p (a b)"), wv_d)
            nc.gpsimd.dma_start(V[:, :, :, DH:DH + 1], vones_d)
            nc.gpsimd.dma_start(WO.rearrange("p a b -> p (a b)"), wo_d)
            import concourse.bass as bass
            for h in range(NH):
                zb = (h % 2) * 64 ^ 64
                zsrc = bass.AP(tensor=zeros_d.tensor, offset=0,
                               ap=[[0, 64], [1, S]])
                nc.gpsimd.dma_start(KT[zb:zb + 64, h, :], zsrc)

            xt_pool = tc.tile_pool(name="xt", bufs=1)
            xt_ctx = xt_pool.__enter__()
            stg_pool = tc.tile_pool(name="stg", bufs=4)
            stg_ctx = stg_pool.__enter__()
            XT = xt_ctx.tile([P, KCH, S], f32r)
            for ch in range(KCH):
                for hh in range(2):
                    stg = stg_ctx.tile([P, 1024], f32, tag="stg",
                                       name=f"stg_{ch}_{hh}")
                    eng = nc.sync if (2 * ch + hh) % 2 == 0 else nc.scalar
                    eng.dma_start(stg, xt_d[ch * P:(ch + 1) * P,
                                            hh * 1024:(hh + 1) * 1024])
                    nc.vector.tensor_copy(
                        XT[:, ch, hh * 1024:(hh + 1) * 1024], stg)

            # ---- PE warmup: get HAM to K=8/8 while input DMAs stream ----
            with tc.tile_pool(name="warm_ps", bufs=1, space="PSUM") as warm_ps:
                wps = warm_ps.tile([P, P], mybir.dt.float32)
                for _ in range(36):
                    nc.tensor.matmul(wps, IDEN, IDEN, start=True, stop=True)

            # ---- phase 1: Q^T, K^T, V projections (chunk-major sweeps so
            # the PE starts as soon as the first x^T chunk lands) ----
            with tc.tile_pool(name="qkv_ps", bufs=8, space="PSUM") as qkv_ps:
                for sweep in range(2):           # qc pair (0,1) then (2,3)
                    pst = {}
                    for wi, (W_, B_) in enumerate(((WQ, BQ), (WK, BK))):
                        for t in range(2):
                            for qc in (2 * sweep, 2 * sweep + 1):
                                pst[(wi, t, qc)] = qkv_ps.tile(
                                    [P, 512], mybir.dt.float32, tag="qk",
                                    name=f"qk_{wi}_{t}_{qc}")
                    for ch in range(KCH):
                        for wi, (W_, B_) in enumerate(((WQ, BQ), (WK, BK))):
                            for t in range(2):
                                for qc in (2 * sweep, 2 * sweep + 1):
                                    nc.tensor.matmul(
                                        pst[(wi, t, qc)],
                                        W_[:, ch, t * P:(t + 1) * P],
                                        XT[:, ch, qc * 512:(qc + 1) * 512],
                                        start=(ch == 0), stop=False)
                    for wi, (W_, B_) in enumerate(((WQ, BQ), (WK, BK))):
                        for t in range(2):
                            for qc in (2 * sweep, 2 * sweep + 1):
                                ps = pst[(wi, t, qc)]
                                nc.tensor.matmul(
                                    ps, B_[:, t * P:(t + 1) * P],
                                    ONES[:, qc * 512:(qc + 1) * 512],
                                    start=False, stop=True)
                                sl = slice(qc * 512, (qc + 1) * 512)
                                if wi == 0:
                                    nc.vector.tensor_copy(QT[:, t, sl], ps)
                                else:
                                    nc.vector.tensor_copy(
                                        KT[0:64, 2 * t, sl], ps[0:64, :])
                                    nc.vector.tensor_copy(
                                        KT[64:128, 2 * t + 1, sl], ps[64:128, :])
                # V sweeps: 8 k-tiles at a time, chunk-major (same slots)
                for vs in range(2):
                    psv = [qkv_ps.tile([P, 512], mybir.dt.float32, tag="qk",
                                       name=f"v_{vs}_{i}") for i in range(KCH)]
                    for ch in range(KCH):
                        for i in range(KCH):
                            kt = vs * KCH + i
                            nc.tensor.matmul(
                                psv[i][:, 0:NH * DH],
                                XT[:, ch, kt * P:(kt + 1) * P],
                                WV[:, ch, :], start=(ch == 0), stop=False)
                    for i in range(KCH):
                        kt = vs * KCH + i
                        nc.tensor.matmul(
                            psv[i][:, 0:NH * DH],
                            ONES[:, kt * P:(kt + 1) * P], BV,
                            start=False, stop=True)
                        nc.vector.tensor_copy(V[:, kt, :, 0:DH], psv[i][:, 0:NH * DH])

            stg_pool.__exit__(None, None, None)
            xt_pool.__exit__(None, None, None)

            # ---- phase 2: attention; strips software-pipelined so the PE
            # emits scores(s+1) before AV(s) and never stalls on exp ----
            with tc.tile_pool(name="sc_ps", bufs=2, space="PSUM") as sc_ps, \
                    tc.tile_pool(name="av_ps", bufs=4, space="PSUM") as av_ps, \
                    tc.tile_pool(name="esp", bufs=4) as esp, \
                    tc.tile_pool(name="nrm", bufs=4) as nrm:
                avs = {}

                def emit_scores(h, kb, hf):
                    t, pb = h // 2, (h % 2) * 64
                    k0 = kb * P
                    hstart = hf * 1024
                    qstart = max(k0, hstart)
                    strip_ps = sc_ps.tile([P, 1024], mybir.dt.float32,
                                          name=f"sps_{h}_{kb}_{hf}", tag="sps")
                    strip_sb = esp.tile([P, 1024], f32r,
                                        name=f"ssb_{h}_{kb}_{hf}", tag="ssb")
                    has_diag = k0 >= hstart
                    qpos = qstart
                    while qpos < hstart + 1024:
                        qnext = min(hstart + 1024, (qpos // 512 + 1) * 512)
                        nc.tensor.matmul(
                            strip_ps[:, qpos - hstart:qnext - hstart],
                            KT[:, h, k0:k0 + P],
                            QT[:, t, qpos:qnext],
                            start=True, stop=True)
                        qpos = qnext
                    nc.scalar.activation(
                        strip_sb[:, qstart - hstart:1024],
                        strip_ps[:, qstart - hstart:1024], Exp)
                    if has_diag:
                        dsl = slice(k0 - hstart, k0 - hstart + P)
                        nc.vector.tensor_tensor(
                            strip_sb[:, dsl], strip_sb[:, dsl], TRI, mult)
                    if debug and h == 0 and kb == 0 and hf == 0:
                        nc.gpsimd.dma_start(dbg["es"], strip_sb)
                    return strip_sb

                def emit_av(h, kb, hf, strip_sb):
                    t, pb = h // 2, (h % 2) * 64
                    k0 = kb * P
                    hstart = hf * 1024
                    qstart = max(k0, hstart)
                    if kb == 0:
                        for qc in (2 * hf, 2 * hf + 1):
                            avs[(h, qc)] = av_ps.tile(
                                [DH + 1, 512], mybir.dt.float32,
                                tag="av", name=f"av_{h}_{qc}")
                    av = {qc: avs[(h, qc)] for qc in (2 * hf, 2 * hf + 1)}
                    qpos = qstart
                    while qpos < hstart + 1024:
                        qc = qpos // 512
                        qnext = min(hstart + 1024, (qc + 1) * 512)
                        done = kb == 4 * qc + 3
                        nc.tensor.matmul(
                            av[qc][:, qpos - qc * 512:qnext - qc * 512],
                            V[:, kb, h, :],
                            strip_sb[:, qpos - hstart:qnext - hstart],
                            start=(kb == 0), stop=done)
                        if done:
                            emit_norm(h, qc, av[qc])
                        qpos = qnext

                def emit_norm(h, qc, avq):
                    t, pb = h // 2, (h % 2) * 64
                    if debug and h == 0:
                        avc = nrm.tile([DH + 1, 512], mybir.dt.float32,
                                       tag="avc", name=f"avc_{qc}")
                        nc.vector.tensor_copy(avc, avq)
                        nc.sync.dma_start(
                            dbg["av"][:, qc * 512:(qc + 1) * 512], avc)
                    rd = nrm.tile([1, 512], mybir.dt.float32, tag="rd")
                    nc.vector.tensor_copy(rd, avq[DH:DH + 1, :])
                    rr = nrm.tile([1, 512], mybir.dt.float32, tag="rr")
                    nc.vector.reciprocal_approx_fast(out=rr, in_=rd)
                    rdb = nrm.tile([64, 512], mybir.dt.float32, tag="rdb")
                    nc.gpsimd.partition_broadcast(rdb, rr)
                    if debug and h == 0:
                        nc.sync.dma_start(
                            dbg["rd"][:, qc * 512:(qc + 1) * 512], rr)
                        nc.sync.dma_start(
                            dbg["rdb"][:, qc * 512:(qc + 1) * 512], rdb)
                    zslc = ZN[pb:pb + 64, t, qc * 512:(qc + 1) * 512]
                    nc.vector.tensor_tensor(zslc, avq[0:DH, :], rdb, mult)

                strip_ids = []
                for h in range(NH):
                    for hf in range(2):
                        for kb in range(NT):
                            if hf * 1024 + 1024 > kb * P:
                                strip_ids.append((h, kb, hf))
                from collections import deque
                pending = deque()
                for sid in strip_ids:
                    sb_tile = emit_scores(*sid)
                    pending.append((sid, sb_tile))
                    if len(pending) > 4:
                        psid, psb = pending.popleft()
                        emit_av(*psid, psb)
                while pending:
                    psid, psb = pending.popleft()
                    emit_av(*psid, psb)

            if debug:
                for nm, tl in (("qt", QT), ("kt", KT), ("v", V), ("zn", ZN)):
                    nc.gpsimd.dma_start(dbg[nm], tl.rearrange("p ... -> p (...)"))

            # ---- phase 3: output projection ----
            with tc.tile_pool(name="op_ps", bufs=3, space="PSUM") as op_ps, \
                    tc.tile_pool(name="osb", bufs=3) as osb:
                for qt in range(NT):
                    for dc in range(2):
                        ps = op_ps.tile([P, 512], mybir.dt.float32)
                        for t in range(2):
                            nc.tensor.matmul(
                                ps, ZN[:, t, qt * P:(qt + 1) * P],
                                WO[:, t, dc * 512:(dc + 1) * 512],
                                start=(t == 0), stop=(t == 1))
                        ob = osb.tile([P, 512], mybir.dt.float32)
                        if (qt + dc) % 2 == 0:
                            nc.scalar.copy(ob, ps)
                        else:
                            nc.vector.tensor_copy(ob, ps)
                        oeng = (nc.sync, nc.gpsimd)[(2 * qt + dc) % 2]
                        oeng.dma_start(
                            out_d[qt * P:(qt + 1) * P, dc * 512:(dc + 1) * 512],
                            ob)

    nc.compile()
    return nc


def _get_nc(debug=False):
    key = ("nc", debug)
    if key not in _CACHE:
        _CACHE[key] = _build_nc(debug)
    return _CACHE[key]


def _host_inputs(x, W_Q, W_K, W_V, W_O, b_Q, b_K, b_V):
    """Build the 8 per-core input maps."""
    x = np.asarray(x, dtype=np.float32)
    scale = 1.0 / np.sqrt(np.float32(DH))
    ones = np.ones((1, S), dtype=np.float32)
    vones = np.ones((P, NT * NH), dtype=np.float32)
    tri = (np.arange(P)[:, None] <= np.arange(P)[None, :]).astype(np.float32)
    trim = np.where(np.arange(P)[:, None] <= np.arange(P)[None, :],
                    np.float32(0.0), np.float32(MASK_VAL)).astype(np.float32)
    iden = np.eye(P, dtype=np.float32)

    xts = [np.ascontiguousarray(x[b].T) for b in range(B)]

    in_maps = []
    for c in range(NCORES):
        b, hg = divmod(c, NCORES // B)
        h0 = NH * hg
        def chunked(a):   # [D, M] -> [128, KCH*M] with rows p, cols (ch, m)
            return np.ascontiguousarray(
                a.reshape(KCH, P, -1).transpose(1, 0, 2).reshape(P, -1))
        wq = chunked((np.asarray(W_Q[h0:h0 + NH], np.float32) * scale)
                     .reshape(NH * DH, D).T)
        wk = chunked(np.asarray(W_K[h0:h0 + NH], np.float32)
                     .reshape(NH * DH, D).T)
        wv = chunked(np.asarray(W_V[h0:h0 + NH], np.float32)
                     .reshape(NH * DH, D).T)
        wo_flat = np.asarray(W_O[h0:h0 + NH], np.float32) \
            .transpose(0, 2, 1).reshape(NH * DH, D)
        wo = np.ascontiguousarray(
            wo_flat.reshape(2, P, D).transpose(1, 0, 2).reshape(P, 2 * D))
        bq = (np.asarray(b_Q[h0:h0 + NH], np.float32) * scale).reshape(1, NH * DH)
        bk = np.asarray(b_K[h0:h0 + NH], np.float32).reshape(1, NH * DH)
        bv = np.asarray(b_V[h0:h0 + NH], np.float32).reshape(1, NH * DH)
        in_maps.append({
            "xt": xts[b], "wq": wq, "wk": wk, "wv": wv, "wo": wo,
            "zeros": np.zeros((1, S), np.float32),
            "bq": np.ascontiguousarray(bq), "bk": np.ascontiguousarray(bk),
            "bv": np.ascontiguousarray(bv), "ones": ones, "vones": vones,
            "tri": tri, "trim": trim, "iden": iden,
        })
    return in_maps


def run_spmd(in_maps, debug=False, **kwargs):
    from concourse import bass_utils
    nc = _get_nc(debug)
    return bass_utils.run_bass_kernel_spmd(
        nc, in_maps, core_ids=list(range(NCORES)), **kwargs)


def kernel(x, W_Q, W_K, W_V, W_O, b_Q, b_K, b_V, b_O):
    in_maps = _host_inputs(x, W_Q, W_K, W_V, W_O, b_Q, b_K, b_V)
    res = run_spmd(in_maps)
    parts = [res.results[c]["out"] for c in range(NCORES)]
    gpb = NCORES // B
    out = np.stack(
        [sum(parts[b * gpb + g] for g in range(gpb)) for b in range(B)], axis=0)
    out += np.asarray(b_O, np.float32)[None, None, :]
    return out.astype(np.float32)
